# revision 7
# baseline (speedup 1.0000x reference)
"""CGCNNConv Trainium2 kernel: 8-core edge-parallel (dst-sorted) implementation.

Math (see problem):
  z = [atom[dst] | atom[src] | edge_feat]           [E, 192]
  y_core   = z @ W_core.T   + b_core                [E, 64]
  y_filter = z @ W_filter.T + b_filter              [E, 64]
  BN over edge axis (training stats, biased var), then
  msg = sigmoid(BN(y_filter)) * softplus(BN(y_core))
  out = atom + segment_sum(msg, dst)

Device strategy per core (atoms sharded contiguously, edges sorted by dst):
  - Prologue: build projection tables P_dst = atom @ W[:, :64].T and
    P_src = atom @ W[:, 64:128].T, each [50176, 128] (128 packed channels:
    0:64 core, 64:128 filter), stored in HBM.
  - Pass 1 (per 128-edge tile): indirect-gather P_dst[dst] then CCE-add
    P_src[src]; matmul edge_feat^T tile against W[:,128:].T (+bias row) into
    PSUM; add -> y tile (fp16), store to HBM; accumulate per-channel
    sum(y), sum(y^2) via mask-vector matmuls into PSUM.
  - AllReduce the [1,256] stats across 8 cores, derive BN scale/shift,
    broadcast to [128,256] via a rank-1 matmul.
  - Pass 2 (per tile): affine + Sigmoid/Softplus + msg product; build the
    one-hot (edge -> group-relative dst) with is_equal vs an iota row; the
    segment sum is a matmul lhsT=onehot, rhs=msg accumulated per 128-atom
    group in PSUM; add atom rows, DMA out.
"""

import os
import sys
import math

import numpy as np

for _p in ("/opt/trn_rl_repo", os.path.expanduser("~/.axon_site/_ro/trn_rl_repo")):
    if os.path.isdir(_p) and _p not in sys.path:
        sys.path.insert(0, _p)

N_ATOMS = 50000
N_EDGES = 800000
D = 64          # node/edge feature dim
C = 128         # packed channels: 0:64 core, 64:128 filter
N_CORES = 8
A_PER_CORE = 6250
GA = 128                       # atoms per scatter group
G_PER_CORE = (A_PER_CORE + GA - 1) // GA      # 49
A_PAD = G_PER_CORE * GA                        # 6272
TILE = 128
CHUNK = 16                     # tiles per DMA/gather chunk
ATOM_PAD = ((N_ATOMS + TILE - 1) // TILE) * TILE   # 50176
BN_EPS = 1e-5

TBL_DT_NP = np.float16         # projection-table dtype (gathered)
EF_DT_NP = np.float16          # edge-feature stream dtype

LAST_EXEC_NS = None


# --------------------------------------------------------------------------
# Host-side preprocessing
# --------------------------------------------------------------------------

def _preprocess(atom_features, edge_features, edge_indices):
    src = np.asarray(edge_indices[:, 0], dtype=np.int64)
    dst = np.asarray(edge_indices[:, 1], dtype=np.int64)
    order = np.argsort(dst, kind="stable")
    dst_s = dst[order]

    # group edge ranges: group (c, g) covers atoms [c*6250 + g*128, ...)
    bounds = []
    for c in range(N_CORES):
        lo = c * A_PER_CORE
        hi = lo + A_PER_CORE
        for g in range(G_PER_CORE):
            bounds.append(min(lo + g * GA, hi))
        bounds.append(hi)
    # searchsorted for group starts per core
    group_edges = np.zeros((N_CORES, G_PER_CORE + 1), dtype=np.int64)
    for c in range(N_CORES):
        lo = c * A_PER_CORE
        hi = lo + A_PER_CORE
        gb = [min(lo + g * GA, hi) for g in range(G_PER_CORE)] + [hi]
        group_edges[c] = np.searchsorted(dst_s, np.array(gb), side="left")

    cnt = group_edges[:, 1:] - group_edges[:, :-1]          # [8, 49]
    T_g = np.maximum(1, (cnt + TILE - 1) // TILE).max(axis=0)  # shared [49]
    NT = int(T_g.sum())
    n_chunks = (NT + CHUNK - 1) // CHUNK
    # pad NT to CHUNK multiple by extending the last group
    pad_tiles = n_chunks * CHUNK - NT
    T_g[-1] += pad_tiles
    NT = n_chunks * CHUNK
    L = NT * TILE
    t_starts = np.concatenate([[0], np.cumsum(T_g)])[:-1]   # tile index of group start

    per_core = []
    for c in range(N_CORES):
        ids = np.full(L, -1, dtype=np.int64)   # original edge id per slot
        for g in range(G_PER_CORE):
            e0, e1 = group_edges[c, g], group_edges[c, g + 1]
            n = e1 - e0
            s = t_starts[g] * TILE
            ids[s:s + n] = order[e0:e1]
        valid = ids >= 0
        idc = np.where(valid, ids, 0)

        ef = np.where(valid[:, None], edge_features[idc], 0.0).astype(np.float32)
        ef_T = np.concatenate([ef.T, np.ones((1, L), np.float32)], axis=0)
        ef_T = ef_T.astype(EF_DT_NP)

        dstv = np.where(valid, dst[idc], 0).astype(np.int32)
        srcv = np.where(valid, src[idc], 0).astype(np.int32)
        # group-relative dst (for one-hot); -1 on padding
        gidx = np.repeat(np.arange(G_PER_CORE), T_g * TILE)
        base = c * A_PER_CORE + gidx * GA
        rel = np.where(valid, dstv - base, -1).astype(np.float16)
        mask = valid.astype(np.float16)

        # [128, NT] layouts (edge t*128+p -> column t, partition p)
        def pm(x):
            return np.ascontiguousarray(x.reshape(NT, TILE).T)

        a0 = c * A_PER_CORE
        arows = np.zeros((A_PAD, D), np.float32)
        n = min(A_PAD, N_ATOMS - a0)
        arows[:n] = atom_features[a0:a0 + n]

        per_core.append({
            "ef_T": np.ascontiguousarray(ef_T),
            "idx_dst": pm(dstv),
            "idx_src": pm(srcv),
            "rel": pm(rel),
            "mask": pm(mask),
            "atom_rows": arows,
        })

    return per_core, T_g.tolist(), t_starts.tolist(), NT, n_chunks


# --------------------------------------------------------------------------
# Device program
# --------------------------------------------------------------------------

def _build_nc(NT, T_g, t_starts, n_chunks):
    import concourse.bass as bass
    import concourse.bacc as bacc
    import concourse.mybir as mybir
    import concourse.tile as tile

    f32 = mybir.dt.float32
    f16 = mybir.dt.float16
    i32 = mybir.dt.int32
    TBL_DT = mybir.dt.float16
    EF_DT = mybir.dt.float16
    ADD = mybir.AluOpType.add
    MUL = mybir.AluOpType.mult
    SUB = mybir.AluOpType.subtract
    EQ = mybir.AluOpType.is_equal
    AF = mybir.ActivationFunctionType

    L = NT * TILE
    nc = bacc.Bacc(None)

    # I/O
    ef_T = nc.dram_tensor("ef_T", [D + 1, L], EF_DT, kind="ExternalInput")
    idx_dst = nc.dram_tensor("idx_dst", [TILE, NT], i32, kind="ExternalInput")
    idx_src = nc.dram_tensor("idx_src", [TILE, NT], i32, kind="ExternalInput")
    rel_d = nc.dram_tensor("rel", [TILE, NT], f16, kind="ExternalInput")
    mask_d = nc.dram_tensor("mask", [TILE, NT], f16, kind="ExternalInput")
    atom_rows = nc.dram_tensor("atom_rows", [A_PAD, D], f32, kind="ExternalInput")
    atom_T = nc.dram_tensor("atom_T", [D, ATOM_PAD], f32, kind="ExternalInput")
    w12T = nc.dram_tensor("w12T", [D, 2 * C], f32, kind="ExternalInput")
    w3b = nc.dram_tensor("w3b", [D + 1, C], EF_DT, kind="ExternalInput")
    iota_d = nc.dram_tensor("iota", [TILE, TILE], f16, kind="ExternalInput")
    ones_d = nc.dram_tensor("ones", [1, C], f32, kind="ExternalInput")
    gb_d = nc.dram_tensor("gb", [1, 2 * C], f32, kind="ExternalInput")
    out_d = nc.dram_tensor("out", [A_PAD, D], f32, kind="ExternalOutput")

    # internal DRAM
    P_dst = nc.dram_tensor("P_dst", [ATOM_PAD, C], TBL_DT)
    P_src = nc.dram_tensor("P_src", [ATOM_PAD, C], TBL_DT)
    y_dram = nc.dram_tensor("y_dram", [TILE, L], f16)
    stats_in = nc.dram_tensor("stats_in", [1, 2 * C], f32)
    stats_out = nc.dram_tensor("stats_out", [1, 2 * C], f32, addr_space="Shared")

    core_ids = list(range(N_CORES))

    with tile.TileContext(nc) as tc:
        with (
            tc.tile_pool(name="const", bufs=1) as const_p,
            tc.tile_pool(name="efp", bufs=3) as ef_p,
            tc.tile_pool(name="gat", bufs=3) as gat_p,
            tc.tile_pool(name="ych", bufs=3) as y_p,
            tc.tile_pool(name="small", bufs=4) as small_p,
            tc.tile_pool(name="qps", bufs=2, space="PSUM") as qps_p,
            tc.tile_pool(name="sps", bufs=1, space="PSUM") as sps_p,
            tc.tile_pool(name="segps", bufs=2, space="PSUM") as seg_p,
            tc.tile_pool(name="ppch", bufs=2) as pp_p,
        ):
            # ---------- resident constants ----------
            w12T_sb = const_p.tile([D, 2 * C], f32)
            nc.sync.dma_start(out=w12T_sb[:], in_=w12T[:])
            w3b_sb = const_p.tile([D + 1, C], EF_DT)
            nc.sync.dma_start(out=w3b_sb[:], in_=w3b[:])
            iota_sb = const_p.tile([TILE, TILE], f16)
            nc.sync.dma_start(out=iota_sb[:], in_=iota_d[:])
            ones_sb = const_p.tile([1, C], f32)
            nc.sync.dma_start(out=ones_sb[:], in_=ones_d[:])
            gb_sb = const_p.tile([1, 2 * C], f32)
            nc.sync.dma_start(out=gb_sb[:], in_=gb_d[:])
            idxd_sb = const_p.tile([TILE, NT], i32)
            nc.sync.dma_start(out=idxd_sb[:], in_=idx_dst[:])
            idxs_sb = const_p.tile([TILE, NT], i32)
            nc.sync.dma_start(out=idxs_sb[:], in_=idx_src[:])
            rel_sb = const_p.tile([TILE, NT], f16)
            nc.sync.dma_start(out=rel_sb[:], in_=rel_d[:])
            mask_sb = const_p.tile([TILE, NT], f16)
            nc.sync.dma_start(out=mask_sb[:], in_=mask_d[:])

            # ---------- prologue: build projection tables ----------
            ACHUNK = 2048
            n_at = ATOM_PAD // TILE          # 392 tiles
            a_done = 0
            while a_done < ATOM_PAD:
                an = min(ACHUNK, ATOM_PAD - a_done)
                ntile = an // TILE
                at_ch = ef_p.tile([D, ACHUNK], f32, tag="atch")
                nc.sync.dma_start(out=at_ch[:, :an], in_=atom_T[:, a_done:a_done + an])
                pp_ch = pp_p.tile([TILE, (ACHUNK // TILE) * 2 * C], TBL_DT)
                for j in range(ntile):
                    pps = qps_p.tile([TILE, 2 * C], f32, space="PSUM", tag="q")
                    nc.tensor.matmul(
                        pps[:],
                        lhsT=at_ch[:, j * TILE:(j + 1) * TILE],
                        rhs=w12T_sb[:],
                        start=True, stop=True,
                    )
                    nc.scalar.copy(
                        out=pp_ch[:, j * 2 * C:(j + 1) * 2 * C], in_=pps[:]
                    )
                pp3 = pp_ch[:].rearrange("p (j c) -> p j c", c=2 * C)
                dst_view = P_dst[a_done:a_done + an, :].rearrange(
                    "(j p) c -> p j c", p=TILE)
                src_view = P_src[a_done:a_done + an, :].rearrange(
                    "(j p) c -> p j c", p=TILE)
                nc.sync.dma_start(out=dst_view, in_=pp3[:, :ntile, 0:C])
                nc.sync.dma_start(out=src_view, in_=pp3[:, :ntile, C:2 * C])
                a_done += an

            # ---------- pass 1 ----------
            stats_y = sps_p.tile([1, C], f32, space="PSUM")
            stats_y2 = sps_p.tile([1, C], f32, space="PSUM")

            for ch in range(n_chunks):
                c0 = ch * CHUNK
                ef_ch = ef_p.tile([D + 1, CHUNK * TILE], EF_DT, tag="efch")
                nc.sync.dma_start(
                    out=ef_ch[:],
                    in_=ef_T[:, c0 * TILE:(c0 + CHUNK) * TILE],
                )
                gat = gat_p.tile([TILE, CHUNK * C], TBL_DT)
                for j in range(CHUNK):
                    t = c0 + j
                    gsl = gat[:, j * C:(j + 1) * C]
                    nc.gpsimd.indirect_dma_start(
                        out=gsl, out_offset=None,
                        in_=P_dst[:],
                        in_offset=bass.IndirectOffsetOnAxis(
                            ap=idxd_sb[:, t:t + 1], axis=0),
                    )
                    nc.gpsimd.indirect_dma_start(
                        out=gsl, out_offset=None,
                        in_=P_src[:],
                        in_offset=bass.IndirectOffsetOnAxis(
                            ap=idxs_sb[:, t:t + 1], axis=0),
                        compute_op=ADD,
                    )
                y_ch = y_p.tile([TILE, CHUNK * TILE], f16)
                for j in range(CHUNK):
                    t = c0 + j
                    qp = qps_p.tile([TILE, C], f32, space="PSUM", tag="q")
                    nc.tensor.matmul(
                        qp[:],
                        lhsT=ef_ch[:, j * TILE:(j + 1) * TILE],
                        rhs=w3b_sb[:],
                        start=True, stop=True,
                    )
                    ysl = y_ch[:, j * C:(j + 1) * C]
                    nc.vector.tensor_tensor(
                        out=ysl, in0=gat[:, j * C:(j + 1) * C], in1=qp[:], op=ADD)
                    y2 = small_p.tile([TILE, C], f16, tag="y2")
                    nc.vector.tensor_tensor(out=y2[:], in0=ysl, in1=ysl, op=MUL)
                    nc.tensor.matmul(
                        stats_y[:], lhsT=mask_sb[:, t:t + 1], rhs=ysl,
                        start=(t == 0), stop=(t == NT - 1))
                    nc.tensor.matmul(
                        stats_y2[:], lhsT=mask_sb[:, t:t + 1], rhs=y2[:],
                        start=(t == 0), stop=(t == NT - 1))
                nc.sync.dma_start(
                    out=y_dram[:, c0 * TILE:(c0 + CHUNK) * TILE], in_=y_ch[:])

            # ---------- BN stats all-reduce + params ----------
            st_sb = small_p.tile([1, 2 * C], f32, tag="st")
            nc.vector.tensor_copy(out=st_sb[:, 0:C], in_=stats_y[:])
            nc.vector.tensor_copy(out=st_sb[:, C:2 * C], in_=stats_y2[:])
            nc.sync.dma_start(out=stats_in[:], in_=st_sb[:])
            nc.gpsimd.collective_compute(
                "AllReduce", ADD,
                replica_groups=[core_ids],
                ins=[stats_in[:]],
                outs=[stats_out[:]],
            )
            stg = small_p.tile([1, 2 * C], f32, tag="stg")
            nc.sync.dma_start(out=stg[:], in_=stats_out[:])

            bn = small_p.tile([1, 5 * C], f32, tag="bn")
            mu = bn[:, 0:C]
            m2 = bn[:, C:2 * C]
            var = bn[:, 2 * C:3 * C]
            sd = bn[:, 3 * C:4 * C]
            inv = bn[:, 4 * C:5 * C]
            inv_e = 1.0 / float(N_EDGES)
            nc.vector.tensor_scalar_mul(mu, stg[:, 0:C], inv_e)
            nc.vector.tensor_scalar_mul(m2, stg[:, C:2 * C], inv_e)
            nc.vector.tensor_tensor(out=var, in0=mu, in1=mu, op=MUL)
            nc.vector.tensor_tensor(out=var, in0=m2, in1=var, op=SUB)
            nc.vector.tensor_scalar_add(var, var, BN_EPS)
            nc.scalar.activation(sd, var, AF.Ln)
            nc.scalar.activation(inv, sd, AF.Exp, scale=-0.5)
            ab = small_p.tile([1, 2 * C], f32, tag="ab")
            nc.vector.tensor_tensor(out=ab[:, 0:C], in0=inv, in1=gb_sb[:, 0:C], op=MUL)
            # bhat = beta - mu * a
            tmp = small_p.tile([1, C], f32, tag="tmp")
            nc.vector.tensor_tensor(out=tmp[:], in0=mu, in1=ab[:, 0:C], op=MUL)
            nc.vector.tensor_tensor(
                out=ab[:, C:2 * C], in0=gb_sb[:, C:2 * C], in1=tmp[:], op=SUB)
            abps = qps_p.tile([TILE, 2 * C], f32, space="PSUM", tag="q")
            nc.tensor.matmul(abps[:], lhsT=ones_sb[:], rhs=ab[:], start=True, stop=True)
            ab_bc = const_p.tile([TILE, 2 * C], f16)
            nc.vector.tensor_copy(out=ab_bc[:], in_=abps[:])

            # ---------- pass 2 ----------
            cur_chunk = [-1, None]

            def get_y(t):
                chi = t // CHUNK
                if cur_chunk[0] != chi:
                    yc = y_p.tile([TILE, CHUNK * TILE], f16, tag="y2ch")
                    nc.sync.dma_start(
                        out=yc[:],
                        in_=y_dram[:, chi * CHUNK * TILE:(chi + 1) * CHUNK * TILE])
                    cur_chunk[0] = chi
                    cur_chunk[1] = yc
                j = t % CHUNK
                return cur_chunk[1][:, j * C:(j + 1) * C]

            for g in range(G_PER_CORE):
                t0 = t_starts[g]
                Tg = T_g[g]
                ps = seg_p.tile([TILE, D], f32, space="PSUM")
                for j in range(Tg):
                    t = t0 + j
                    ysl = get_y(t)
                    yn = small_p.tile([TILE, C], f16, tag="yn")
                    nc.vector.tensor_tensor(
                        out=yn[:], in0=ysl, in1=ab_bc[:, 0:C], op=MUL)
                    nc.vector.tensor_tensor(
                        out=yn[:], in0=yn[:], in1=ab_bc[:, C:2 * C], op=ADD)
                    gs = small_p.tile([TILE, C], f32, tag="gs")
                    sg = gs[:, 0:D]
                    gt = gs[:, D:C]
                    nc.scalar.activation(sg, yn[:, 0:D], AF.Exp)
                    nc.scalar.activation(sg, sg, AF.Ln, bias=1.0)
                    nc.scalar.activation(gt, yn[:, D:C], AF.Exp, scale=-1.0)
                    nc.scalar.activation(gt, gt, AF.Ln, bias=1.0)
                    nc.scalar.activation(gt, gt, AF.Exp, scale=-1.0)
                    msg = small_p.tile([TILE, D], f16, tag="msg")
                    nc.vector.tensor_tensor(out=msg[:], in0=sg, in1=gt, op=MUL)
                    oh = small_p.tile([TILE, TILE], f16, tag="oh")
                    nc.vector.tensor_tensor(
                        out=oh[:],
                        in0=rel_sb[:, t:t + 1].to_broadcast([TILE, TILE]),
                        in1=iota_sb[:], op=EQ)
                    nc.tensor.matmul(
                        ps[:], lhsT=oh[:], rhs=msg[:],
                        start=(j == 0), stop=(j == Tg - 1))
                at = small_p.tile([TILE, D], f32, tag="at")
                nc.sync.dma_start(
                    out=at[:], in_=atom_rows[g * GA:(g + 1) * GA, :])
                ot = small_p.tile([TILE, D], f32, tag="ot")
                nc.vector.tensor_tensor(out=ot[:], in0=ps[:], in1=at[:], op=ADD)
                nc.sync.dma_start(out=out_d[g * GA:(g + 1) * GA, :], in_=ot[:])

    nc.finalize()
    return nc


# --------------------------------------------------------------------------
# Entry point
# --------------------------------------------------------------------------

def kernel(atom_features, edge_features, W_filter, b_filter, gamma_filter,
           beta_filter, W_core, b_core, gamma_core, beta_core, edge_indices):
    global LAST_EXEC_NS
    from concourse.bass_utils import run_bass_kernel_spmd

    atom_features = np.asarray(atom_features, np.float32)
    edge_features = np.asarray(edge_features, np.float32)

    per_core, T_g, t_starts, NT, n_chunks = _preprocess(
        atom_features, edge_features, np.asarray(edge_indices))

    W_all = np.vstack([np.asarray(W_core, np.float32),
                       np.asarray(W_filter, np.float32)])       # [128, 192]
    b_all = np.concatenate([np.asarray(b_core, np.float32),
                            np.asarray(b_filter, np.float32)])  # [128]
    gamma_all = np.concatenate([np.asarray(gamma_core, np.float32),
                                np.asarray(gamma_filter, np.float32)])
    beta_all = np.concatenate([np.asarray(beta_core, np.float32),
                               np.asarray(beta_filter, np.float32)])

    atom_T = np.zeros((D, ATOM_PAD), np.float32)
    atom_T[:, :N_ATOMS] = atom_features.T
    w12T = np.concatenate([W_all[:, 0:D].T, W_all[:, D:2 * D].T],
                          axis=1).astype(np.float32)            # [64, 256]
    w3b = np.concatenate([W_all[:, 2 * D:3 * D].T, b_all[None, :]],
                         axis=0).astype(EF_DT_NP)               # [65, 128]
    iota = np.broadcast_to(np.arange(TILE, dtype=np.float16), (TILE, TILE))
    ones_row = np.ones((1, C), np.float32)
    gb = np.concatenate([gamma_all, beta_all])[None, :].astype(np.float32)  # [1, 256]

    shared = {
        "atom_T": atom_T,
        "w12T": w12T,
        "w3b": np.ascontiguousarray(w3b),
        "iota": np.ascontiguousarray(iota),
        "ones": ones_row,
        "gb": gb,
    }
    in_maps = []
    for c in range(N_CORES):
        m = dict(shared)
        m.update(per_core[c])
        in_maps.append(m)

    nc = _build_nc(NT, T_g, t_starts, n_chunks)

    trace = bool(int(os.environ.get("KERNEL_TRACE", "0")))
    res = run_bass_kernel_spmd(nc, in_maps, list(range(N_CORES)), trace=trace)
    LAST_EXEC_NS = res.exec_time_ns

    out = np.zeros((N_ATOMS, D), np.float32)
    for c in range(N_CORES):
        n = min(A_PER_CORE, N_ATOMS - c * A_PER_CORE)
        out[c * A_PER_CORE:c * A_PER_CORE + n] = res.results[c]["out"][:n]
    return out
